# revision 13
# baseline (speedup 1.0000x reference)
"""BiLSTM-CRF loss kernel for 8 Trainium2 NeuronCores.

Sharding: direction x batch split. Cores 0-3 run the forward LSTM on batch
slices of 16 sequences; cores 4-7 run the backward LSTM (token order reversed
via the gather indices). The embedding table and all weights live device-side
(re-uploaded only if their values change); each warm call ships just the int32
token indices (~256 KB), runs one cached jit dispatch, and pulls back partial
emission features. Embedding gather runs on-device via gpsimd
indirect_dma_start (128 rows per DMA, one row index per partition), followed
by PE-array transposes into the [E, tokens] layout the input projection
needs. The tiny CRF runs on host.
"""

import numpy as np
import ml_dtypes

import concourse.bass as bass
import concourse.mybir as mybir
import concourse.tile as tile
from concourse import bacc

BF16 = ml_dtypes.bfloat16

B, L, V, E, HD, T = 64, 512, 32000, 512, 1024, 10
H = HD // 2          # 512 per-direction hidden
G4 = 4 * H           # 2048 gate rows
BL = 16              # sequences per core (64 batch / 4 slices; dirs split 0-3/4-7)
NC = L * BL          # 8192 (t-major columns: col = t*BL + b)
KC = H // 128        # 4 contraction chunks
MC = G4 // 128       # 16 gate-row chunks
NB = NC // 512       # 16 column blocks for the input projection
JB = NC // 128       # 64 gather blocks of 128 tokens
SB = 8               # recurrence steps per pre-gate prefetch block

F32 = mybir.dt.float32
BF16_T = mybir.dt.bfloat16
I32 = mybir.dt.int32
AF = mybir.ActivationFunctionType

_state = {}


def _build_program(steps=L):
    nc = bacc.Bacc("TRN2", target_bir_lowering=False, debug=False, num_devices=8)

    emb = nc.dram_tensor("emb", [V, E], BF16_T, kind="ExternalInput").ap()
    idxs = nc.dram_tensor("idxs", [128, JB], I32, kind="ExternalInput").ap()
    ident = nc.dram_tensor("ident", [128, 128], BF16_T, kind="ExternalInput").ap()
    w_ihT = nc.dram_tensor("w_ihT", [E, G4], BF16_T, kind="ExternalInput").ap()
    w_hhT = nc.dram_tensor("w_hhT", [H, G4], BF16_T, kind="ExternalInput").ap()
    bias_pm = nc.dram_tensor("bias_pm", [128, MC], F32, kind="ExternalInput").ap()
    w_outT = nc.dram_tensor("w_outT", [H, T], BF16_T, kind="ExternalInput").ap()
    feats = nc.dram_tensor("feats", [T, NC], F32, kind="ExternalOutput").ap()
    pre = nc.dram_tensor("pre", [MC, 128, NC], F32).ap()  # scratch in DRAM

    with tile.TileContext(nc) as tc:
        with (
            tc.tile_pool(name="singles", bufs=1) as singles,
            tc.tile_pool(name="xin", bufs=1) as xin,
            tc.tile_pool(name="xn", bufs=4) as xnp,
            tc.tile_pool(name="psT", bufs=2, space="PSUM") as psTp,
            tc.tile_pool(name="psA", bufs=2, space="PSUM") as psA,
            tc.tile_pool(name="evA", bufs=4) as evA,
            tc.tile_pool(name="prestream", bufs=2) as prestream,
            tc.tile_pool(name="psB", bufs=2, space="PSUM") as psB,
            tc.tile_pool(name="gtmp", bufs=2) as gtmp,
            tc.tile_pool(name="atmp", bufs=2) as atmp,
            tc.tile_pool(name="stmp", bufs=3) as stmp,
            tc.tile_pool(name="psF", bufs=2, space="PSUM") as psFp,
            tc.tile_pool(name="evF", bufs=2) as evFp,
        ):
            # ---- resident weights ----
            wih_sb = [singles.tile([128, G4], BF16_T, tag=f"wih{k}", name=f"wih{k}") for k in range(KC)]
            whh_sb = [singles.tile([128, G4], BF16_T, tag=f"whh{k}", name=f"whh{k}") for k in range(KC)]
            for k in range(KC):
                nc.sync.dma_start(out=wih_sb[k], in_=w_ihT[128 * k:128 * (k + 1), :])
                nc.sync.dma_start(out=whh_sb[k], in_=w_hhT[128 * k:128 * (k + 1), :])
            bias_sb = singles.tile([128, MC], F32, tag="bias")
            nc.sync.dma_start(out=bias_sb, in_=bias_pm)
            wout_sb = [singles.tile([128, T], BF16_T, tag=f"wo{k}", name=f"wo{k}") for k in range(KC)]
            for k in range(KC):
                nc.sync.dma_start(out=wout_sb[k], in_=w_outT[128 * k:128 * (k + 1), :])

            # ---- on-device embedding gather + PE transpose to [E, tokens] ----
            idx_sb = singles.tile([128, JB], I32, tag="idx")
            nc.sync.dma_start(out=idx_sb, in_=idxs)
            ident_sb = singles.tile([128, 128], BF16_T, tag="ident")
            nc.sync.dma_start(out=ident_sb, in_=ident)

            xT = xin.tile([128, KC, NC], BF16_T, tag="xT")
            for j in range(JB):
                xn = xnp.tile([128, E], BF16_T)
                nc.gpsimd.indirect_dma_start(
                    out=xn, out_offset=None, in_=emb,
                    in_offset=bass.IndirectOffsetOnAxis(
                        ap=idx_sb[:, j:j + 1], axis=0),
                )
                for k in range(KC):
                    psT = psTp.tile([128, 128], BF16_T)
                    nc.tensor.transpose(psT, xn[:, 128 * k:128 * (k + 1)], ident_sb)
                    nc.vector.tensor_copy(xT[:, k, 128 * j:128 * (j + 1)], psT)

            # ---- phase A: pre-gates = W_ih @ x (+bias), streamed to DRAM ----
            for m in range(MC):
                for nb in range(NB):
                    ps = psA.tile([128, 512], F32)
                    for k in range(KC):
                        nc.tensor.matmul(
                            ps,
                            wih_sb[k][:, 128 * m:128 * (m + 1)],
                            xT[:, k, 512 * nb:512 * (nb + 1)],
                            start=(k == 0), stop=(k == KC - 1),
                        )
                    ev = evA.tile([128, 512], F32)
                    nc.scalar.activation(ev, ps, AF.Identity,
                                         bias=bias_sb[:, m:m + 1])
                    nc.sync.dma_start(out=pre[m, :, 512 * nb:512 * (nb + 1)], in_=ev)

            # ---- phase B: recurrence ----
            # h history: [128, KC, (steps+1)*BL] bf16; col block s holds h_{s-1}
            hh = singles.tile([128, KC, (steps + 1) * BL], BF16_T, tag="hh")
            nc.vector.memset(hh[:, :, 0:BL], 0.0)
            c_sb = singles.tile([128, KC, BL], F32, tag="c")
            nc.vector.memset(c_sb, 0.0)

            preT = pre.rearrange("m p c -> p m c")
            for tb in range(steps // SB):
                pt = prestream.tile([128, MC, SB * BL], F32)
                nc.sync.dma_start(
                    out=pt, in_=preT[:, :, BL * SB * tb:BL * SB * (tb + 1)])
                for ts in range(SB):
                    t = tb * SB + ts
                    ptt = pt[:, :, BL * ts:BL * (ts + 1)]     # [128, MC, BL]
                    ps = psB.tile([128, MC, BL], F32)
                    hprev = hh[:, :, BL * t:BL * (t + 1)]     # [128, KC, BL]
                    for m in range(MC):
                        for k in range(KC):
                            nc.tensor.matmul(
                                ps[:, m, :],
                                whh_sb[k][:, 128 * m:128 * (m + 1)],
                                hprev[:, k, :],
                                start=(k == 0), stop=(k == KC - 1),
                            )
                    g_sb = gtmp.tile([128, MC, BL], F32)
                    # i,f block ready after m=7; g,o after m=15
                    nc.vector.tensor_add(g_sb[:, 0:8, :], ps[:, 0:8, :], ptt[:, 0:8, :])
                    nc.vector.tensor_add(g_sb[:, 8:16, :], ps[:, 8:16, :], ptt[:, 8:16, :])
                    a_sb = atmp.tile([128, MC, BL], F32)
                    nc.scalar.activation(a_sb[:, 0:8, :], g_sb[:, 0:8, :], AF.Sigmoid)
                    nc.scalar.activation(a_sb[:, 8:12, :], g_sb[:, 8:12, :], AF.Tanh)
                    nc.scalar.activation(a_sb[:, 12:16, :], g_sb[:, 12:16, :], AF.Sigmoid)
                    t1 = stmp.tile([128, KC, BL], F32, tag="t1")
                    nc.vector.tensor_mul(t1, a_sb[:, 0:4, :], a_sb[:, 8:12, :])
                    nc.vector.tensor_mul(c_sb, a_sb[:, 4:8, :], c_sb)
                    nc.vector.tensor_add(c_sb, c_sb, t1)
                    tcn = stmp.tile([128, KC, BL], F32, tag="tc")
                    nc.scalar.activation(tcn, c_sb, AF.Tanh)
                    hout = hh[:, :, BL * (t + 1):BL * (t + 2)]
                    nc.vector.tensor_mul(hout, a_sb[:, 12:16, :], tcn)

            # ---- phase C: partial feats = w_out_half.T @ h ----
            ncols_h = steps * BL
            cblk = min(512, ncols_h)
            for nb in range(ncols_h // cblk):
                psF = psFp.tile([T, cblk], F32)
                for k in range(KC):
                    nc.tensor.matmul(
                        psF,
                        wout_sb[k],
                        hh[:, k, BL + cblk * nb:BL + cblk * (nb + 1)],
                        start=(k == 0), stop=(k == KC - 1),
                    )
                evF = evFp.tile([T, cblk], F32)
                nc.vector.tensor_copy(evF, psF)
                nc.sync.dma_start(out=feats[:, cblk * nb:cblk * (nb + 1)], in_=evF)

    nc.compile()
    return nc


# ---------------------------------------------------------------------------
# host-side helpers
# ---------------------------------------------------------------------------

def _to_bf16_u16(a):
    """f32 ndarray -> uint16 bf16 bits, round-to-nearest-even (matches ml_dtypes)."""
    a = np.ascontiguousarray(a, np.float32)
    u = a.view(np.uint32)
    return ((u + np.uint32(0x7FFF) + ((u >> np.uint32(16)) & np.uint32(1)))
            >> np.uint32(16)).astype(np.uint16)


def _bf16(a):
    return _to_bf16_u16(a).view(BF16)


def _same(a, b):
    return a is b or (a.shape == b.shape and a.dtype == b.dtype
                      and np.array_equal(a, b))


def _make_runner(nc):
    import jax
    from jax.sharding import Mesh, PartitionSpec
    try:
        from jax.experimental.shard_map import shard_map
    except ImportError:
        from jax.shard_map import shard_map
    from concourse import bass2jax

    bass2jax.install_neuronx_cc_hook()

    partition_name = (nc.partition_id_tensor.name
                      if nc.partition_id_tensor is not None else None)

    in_names, out_names, out_avals, zero_outs = [], [], [], []
    for alloc in nc.m.functions[0].allocations:
        if not isinstance(alloc, mybir.MemoryLocationSet):
            continue
        name = alloc.memorylocations[0].name
        if alloc.kind == "ExternalInput":
            if name != partition_name:
                in_names.append(name)
        elif alloc.kind == "ExternalOutput":
            shape = tuple(alloc.tensor_shape)
            dtype = mybir.dt.np(alloc.dtype)
            out_names.append(name)
            out_avals.append(jax.core.ShapedArray(shape, dtype))
            zero_outs.append(np.zeros(shape, dtype))
    n_params = len(in_names)
    n_outs = len(out_avals)
    all_names = list(in_names) + list(out_names)
    if partition_name is not None:
        all_names.append(partition_name)

    donate = tuple(range(n_params, n_params + n_outs))

    def _body(*args):
        operands = list(args)
        if partition_name is not None:
            operands.append(bass2jax.partition_id_tensor())
        outs = bass2jax._bass_exec_p.bind(
            *operands,
            out_avals=tuple(out_avals),
            in_names=tuple(all_names),
            out_names=tuple(out_names),
            lowering_input_output_aliases=(),
            sim_require_finite=True,
            sim_require_nnan=True,
            nc=nc,
        )
        return tuple(outs)

    devices = jax.devices()[:8]
    mesh = Mesh(np.asarray(devices), ("core",))
    # emb is replicated (every core gathers from the full table); everything
    # else is concatenated per-core along axis 0.
    def spec_for(name):
        return PartitionSpec() if name in ("emb", "ident") else PartitionSpec("core")
    in_specs = tuple(spec_for(n) for n in in_names) + \
        tuple(PartitionSpec("core") for _ in range(n_outs))
    out_specs = tuple(PartitionSpec("core") for _ in out_names)
    sharded = jax.jit(
        shard_map(_body, mesh=mesh, in_specs=in_specs, out_specs=out_specs,
                  check_rep=False),
        donate_argnums=donate, keep_unused=True,
    )
    return {
        "fn": sharded,
        "mesh": mesh,
        "in_names": in_names,
        "out_names": out_names,
        "zero_shapes": [(tuple(a.shape), a.dtype) for a in out_avals],
    }


def _ensure_weights(runner, inputs):
    """Upload emb + per-direction weights if changed; return device arg dict."""
    import jax
    from jax.sharding import NamedSharding, PartitionSpec

    keys = ("emb", "w_ih_f", "w_hh_f", "b_f", "w_ih_b", "w_hh_b", "b_b", "w_out")
    cached = _state.get("weight_cache")
    if cached is not None and all(_same(inputs[k], cached["host"][k]) for k in keys):
        return cached["dev"]

    mesh = runner["mesh"]
    emb_bf = _bf16(inputs["emb"])                       # [V, E]

    def cat(parts_u16):
        return np.concatenate(parts_u16, axis=0)

    wih = {}
    whh = {}
    wout = {}
    bias = {}
    for d, (wi, wh, bb) in enumerate(
            [(inputs["w_ih_f"], inputs["w_hh_f"], inputs["b_f"]),
             (inputs["w_ih_b"], inputs["w_hh_b"], inputs["b_b"])]):
        wih[d] = _to_bf16_u16(np.ascontiguousarray(np.asarray(wi, np.float32).T))
        whh[d] = _to_bf16_u16(np.ascontiguousarray(np.asarray(wh, np.float32).T))
        bias[d] = np.ascontiguousarray(
            np.asarray(bb, np.float32).reshape(MC, 128).T)
    w_out = np.asarray(inputs["w_out"], np.float32)
    wout[0] = _to_bf16_u16(np.ascontiguousarray(w_out[:, :H].T))
    wout[1] = _to_bf16_u16(np.ascontiguousarray(w_out[:, H:].T))

    host_args = {
        "emb": emb_bf,
        "ident": np.eye(128, dtype=np.float32).astype(BF16),
        "w_ihT": cat([wih[0]] * 4 + [wih[1]] * 4).view(BF16),
        "w_hhT": cat([whh[0]] * 4 + [whh[1]] * 4).view(BF16),
        "bias_pm": np.concatenate([bias[0]] * 4 + [bias[1]] * 4, axis=0),
        "w_outT": cat([wout[0]] * 4 + [wout[1]] * 4).view(BF16),
    }
    dev = {}
    for name, arr in host_args.items():
        spec = PartitionSpec() if name in ("emb", "ident") else PartitionSpec("core")
        dev[name] = jax.device_put(arr, NamedSharding(mesh, spec))
    for a in dev.values():
        a.block_until_ready()
    _state["weight_cache"] = {
        "host": {k: inputs[k] for k in keys},
        "dev": dev,
    }
    return dev


def _logsumexp(a, axis):
    m = np.max(a, axis=axis, keepdims=True)
    return (m + np.log(np.sum(np.exp(a - m), axis=axis, keepdims=True))).squeeze(axis)


def _host_crf(feats, tags, mask, b_out, start_trans, end_trans, transitions):
    feats = feats + np.asarray(b_out, np.float32)[None, None, :]
    trans = np.asarray(transitions, np.float32)
    start = np.asarray(start_trans, np.float32)
    end = np.asarray(end_trans, np.float32)
    maskT = mask.T.astype(np.float32)       # [L, B]
    tagsT = tags.T                          # [L, B]
    em = np.take_along_axis(feats, tagsT[:, :, None], axis=2)[..., 0]  # [L, B]
    score = start[tagsT[0]] + em[0]
    tr = trans[tagsT[:-1], tagsT[1:]]
    score = score + ((tr + em[1:]) * maskT[1:]).sum(axis=0)
    last = mask.sum(axis=1).astype(np.int64) - 1
    last_tags = np.take_along_axis(tags, last[:, None], axis=1)[:, 0]
    score = score + end[last_tags]

    alpha = start[None, :] + feats[0]
    trb = trans[None, :, :]
    all_ones = bool(mask.all())
    for t in range(1, L):
        tmp = alpha[:, :, None] + trb
        tmp += feats[t][:, None, :]
        m = tmp.max(axis=1)
        np.exp(tmp - m[:, None, :], out=tmp)
        nxt = m + np.log(tmp.sum(axis=1))
        if all_ones:
            alpha = nxt
        else:
            alpha = np.where(maskT[t][:, None] > 0, nxt, alpha)
    denom = _logsumexp(alpha + end[None, :], axis=1)
    llh = score - denom
    return -(llh.sum() / maskT.sum())


def kernel(sentence, tags, mask, emb, w_ih_f, w_hh_f, b_f,
           w_ih_b, w_hh_b, b_b, w_out, b_out,
           start_trans, end_trans, transitions):
    sentence = np.asarray(sentence)
    tags = np.asarray(tags)
    mask = np.asarray(mask)

    if "nc" not in _state:
        _state["nc"] = _build_program()
        _state["runner"] = _make_runner(_state["nc"])
    runner = _state["runner"]

    dev = _ensure_weights(runner, {
        "emb": np.asarray(emb), "w_ih_f": np.asarray(w_ih_f),
        "w_hh_f": np.asarray(w_hh_f), "b_f": np.asarray(b_f),
        "w_ih_b": np.asarray(w_ih_b), "w_hh_b": np.asarray(w_hh_b),
        "b_b": np.asarray(b_b), "w_out": np.asarray(w_out),
    })

    # ---- per-call: token indices, t-major, reversed for backward cores ----
    # idx[p, j] = token id at t-major position j*128 + p
    idx_all = np.empty((8, 128, JB), np.int32)
    for c in range(8):
        sl = sentence[(c % 4) * BL:(c % 4) * BL + BL]   # [BL, L]
        tm = sl.T if c < 4 else sl.T[::-1]              # [L, BL] t-major
        flat = np.ascontiguousarray(tm).reshape(-1)
        idx_all[c] = flat.reshape(JB, 128).T
    idx_concat = idx_all.reshape(8 * 128, JB)

    args = {"idxs": idx_concat, **dev}
    ordered = [args[n] for n in runner["in_names"]]
    zeros = [np.zeros((8 * s[0], *s[1:]), d) for s, d in runner["zero_shapes"]]
    outs = runner["fn"](*ordered, *zeros)
    feats_cat = np.asarray(outs[runner["out_names"].index("feats")])

    # ---- assemble full feats [L, B, T] ----
    f = feats_cat.reshape(8, T, L, BL).transpose(0, 2, 3, 1)  # [8, L, BL, T]
    feats_full = np.empty((L, B, T), np.float32)
    for s in range(4):
        feats_full[:, s * BL:(s + 1) * BL, :] = f[s] + f[4 + s, ::-1]

    loss = _host_crf(feats_full, tags, mask, b_out, start_trans, end_trans,
                     transitions)
    return np.float32(loss)


# revision 14
# speedup vs baseline: 1.2684x; 1.2684x over previous
"""BiLSTM-CRF loss kernel for 8 Trainium2 NeuronCores.

Sharding: direction x batch split. Cores 0-3 run the forward LSTM on batch
slices of 16 sequences; cores 4-7 run the backward LSTM (token order reversed
via the gather indices). The embedding table and all weights live device-side
(re-uploaded only if their values change); each warm call ships just the int32
token indices (~256 KB), runs one cached jit dispatch, and pulls back partial
emission features. Embedding gather runs on-device via gpsimd
indirect_dma_start (128 rows per DMA, one row index per partition), followed
by PE-array transposes into the [E, tokens] layout the input projection
needs. The tiny CRF runs on host.
"""

import os
import time
import numpy as np
import ml_dtypes

import concourse.bass as bass
import concourse.mybir as mybir
import concourse.tile as tile
from concourse import bacc

BF16 = ml_dtypes.bfloat16

B, L, V, E, HD, T = 64, 512, 32000, 512, 1024, 10
H = HD // 2          # 512 per-direction hidden
G4 = 4 * H           # 2048 gate rows
BL = 16              # sequences per core (64 batch / 4 slices; dirs split 0-3/4-7)
NC = L * BL          # 8192 (t-major columns: col = t*BL + b)
KC = H // 128        # 4 contraction chunks
MC = G4 // 128       # 16 gate-row chunks
NB = NC // 512       # 16 column blocks for the input projection
JB = NC // 128       # 64 gather blocks of 128 tokens
SB = 8               # recurrence steps per pre-gate prefetch block

F32 = mybir.dt.float32
BF16_T = mybir.dt.bfloat16
I32 = mybir.dt.int32
AF = mybir.ActivationFunctionType

_state = {}


def _build_program(steps=L):
    nc = bacc.Bacc("TRN2", target_bir_lowering=False, debug=False, num_devices=8)

    emb = nc.dram_tensor("emb", [V, E], BF16_T, kind="ExternalInput").ap()
    idxs = nc.dram_tensor("idxs", [128, JB], I32, kind="ExternalInput").ap()
    ident = nc.dram_tensor("ident", [128, 128], BF16_T, kind="ExternalInput").ap()
    w_ihT = nc.dram_tensor("w_ihT", [E, G4], BF16_T, kind="ExternalInput").ap()
    w_hhT = nc.dram_tensor("w_hhT", [H, G4], BF16_T, kind="ExternalInput").ap()
    bias_pm = nc.dram_tensor("bias_pm", [128, MC], F32, kind="ExternalInput").ap()
    w_outT = nc.dram_tensor("w_outT", [H, T], BF16_T, kind="ExternalInput").ap()
    feats = nc.dram_tensor("feats", [T, NC], F32, kind="ExternalOutput").ap()
    pre = nc.dram_tensor("pre", [MC, 128, NC], F32).ap()  # scratch in DRAM

    with tile.TileContext(nc) as tc:
        with (
            tc.tile_pool(name="singles", bufs=1) as singles,
            tc.tile_pool(name="xin", bufs=1) as xin,
            tc.tile_pool(name="xn", bufs=4) as xnp,
            tc.tile_pool(name="psT", bufs=2, space="PSUM") as psTp,
            tc.tile_pool(name="psA", bufs=2, space="PSUM") as psA,
            tc.tile_pool(name="evA", bufs=4) as evA,
            tc.tile_pool(name="prestream", bufs=2) as prestream,
            tc.tile_pool(name="psB", bufs=2, space="PSUM") as psB,
            tc.tile_pool(name="gtmp", bufs=2) as gtmp,
            tc.tile_pool(name="atmp", bufs=2) as atmp,
            tc.tile_pool(name="stmp", bufs=3) as stmp,
            tc.tile_pool(name="psF", bufs=2, space="PSUM") as psFp,
            tc.tile_pool(name="evF", bufs=2) as evFp,
        ):
            # ---- resident weights ----
            wih_sb = [singles.tile([128, G4], BF16_T, tag=f"wih{k}", name=f"wih{k}") for k in range(KC)]
            whh_sb = [singles.tile([128, G4], BF16_T, tag=f"whh{k}", name=f"whh{k}") for k in range(KC)]
            for k in range(KC):
                nc.sync.dma_start(out=wih_sb[k], in_=w_ihT[128 * k:128 * (k + 1), :])
                nc.sync.dma_start(out=whh_sb[k], in_=w_hhT[128 * k:128 * (k + 1), :])
            bias_sb = singles.tile([128, MC], F32, tag="bias")
            nc.sync.dma_start(out=bias_sb, in_=bias_pm)
            wout_sb = [singles.tile([128, T], BF16_T, tag=f"wo{k}", name=f"wo{k}") for k in range(KC)]
            for k in range(KC):
                nc.sync.dma_start(out=wout_sb[k], in_=w_outT[128 * k:128 * (k + 1), :])

            # ---- on-device embedding gather + PE transpose to [E, tokens] ----
            idx_sb = singles.tile([128, JB], I32, tag="idx")
            nc.sync.dma_start(out=idx_sb, in_=idxs)
            ident_sb = singles.tile([128, 128], BF16_T, tag="ident")
            nc.sync.dma_start(out=ident_sb, in_=ident)

            xT = xin.tile([128, KC, NC], BF16_T, tag="xT")
            for j in range(JB):
                xn = xnp.tile([128, E], BF16_T)
                nc.gpsimd.indirect_dma_start(
                    out=xn, out_offset=None, in_=emb,
                    in_offset=bass.IndirectOffsetOnAxis(
                        ap=idx_sb[:, j:j + 1], axis=0),
                )
                for k in range(KC):
                    psT = psTp.tile([128, 128], BF16_T)
                    nc.tensor.transpose(psT, xn[:, 128 * k:128 * (k + 1)], ident_sb)
                    nc.vector.tensor_copy(xT[:, k, 128 * j:128 * (j + 1)], psT)

            # ---- phase A: pre-gates = W_ih @ x (+bias), streamed to DRAM ----
            for m in range(MC):
                for nb in range(NB):
                    ps = psA.tile([128, 512], F32)
                    for k in range(KC):
                        nc.tensor.matmul(
                            ps,
                            wih_sb[k][:, 128 * m:128 * (m + 1)],
                            xT[:, k, 512 * nb:512 * (nb + 1)],
                            start=(k == 0), stop=(k == KC - 1),
                        )
                    ev = evA.tile([128, 512], F32)
                    nc.scalar.activation(ev, ps, AF.Identity,
                                         bias=bias_sb[:, m:m + 1])
                    nc.sync.dma_start(out=pre[m, :, 512 * nb:512 * (nb + 1)], in_=ev)

            # ---- phase B: recurrence ----
            # h history: [128, KC, (steps+1)*BL] bf16; col block s holds h_{s-1}
            hh = singles.tile([128, KC, (steps + 1) * BL], BF16_T, tag="hh")
            nc.vector.memset(hh[:, :, 0:BL], 0.0)
            c_sb = singles.tile([128, KC, BL], F32, tag="c")
            nc.vector.memset(c_sb, 0.0)

            preT = pre.rearrange("m p c -> p m c")
            for tb in range(steps // SB):
                pt = prestream.tile([128, MC, SB * BL], F32)
                nc.sync.dma_start(
                    out=pt, in_=preT[:, :, BL * SB * tb:BL * SB * (tb + 1)])
                for ts in range(SB):
                    t = tb * SB + ts
                    ptt = pt[:, :, BL * ts:BL * (ts + 1)]     # [128, MC, BL]
                    ps = psB.tile([128, MC, BL], F32)
                    hprev = hh[:, :, BL * t:BL * (t + 1)]     # [128, KC, BL]
                    for m in range(MC):
                        for k in range(KC):
                            nc.tensor.matmul(
                                ps[:, m, :],
                                whh_sb[k][:, 128 * m:128 * (m + 1)],
                                hprev[:, k, :],
                                start=(k == 0), stop=(k == KC - 1),
                            )
                    g_sb = gtmp.tile([128, MC, BL], F32)
                    # i,f block ready after m=7; g,o after m=15
                    nc.vector.tensor_add(g_sb[:, 0:8, :], ps[:, 0:8, :], ptt[:, 0:8, :])
                    nc.vector.tensor_add(g_sb[:, 8:16, :], ps[:, 8:16, :], ptt[:, 8:16, :])
                    a_sb = atmp.tile([128, MC, BL], F32)
                    nc.scalar.activation(a_sb[:, 0:8, :], g_sb[:, 0:8, :], AF.Sigmoid)
                    nc.scalar.activation(a_sb[:, 8:12, :], g_sb[:, 8:12, :], AF.Tanh)
                    nc.scalar.activation(a_sb[:, 12:16, :], g_sb[:, 12:16, :], AF.Sigmoid)
                    t1 = stmp.tile([128, KC, BL], F32, tag="t1")
                    nc.vector.tensor_mul(t1, a_sb[:, 0:4, :], a_sb[:, 8:12, :])
                    nc.vector.tensor_mul(c_sb, a_sb[:, 4:8, :], c_sb)
                    nc.vector.tensor_add(c_sb, c_sb, t1)
                    tcn = stmp.tile([128, KC, BL], F32, tag="tc")
                    nc.scalar.activation(tcn, c_sb, AF.Tanh)
                    hout = hh[:, :, BL * (t + 1):BL * (t + 2)]
                    nc.vector.tensor_mul(hout, a_sb[:, 12:16, :], tcn)

            # ---- phase C: partial feats = w_out_half.T @ h ----
            ncols_h = steps * BL
            cblk = min(512, ncols_h)
            for nb in range(ncols_h // cblk):
                psF = psFp.tile([T, cblk], F32)
                for k in range(KC):
                    nc.tensor.matmul(
                        psF,
                        wout_sb[k],
                        hh[:, k, BL + cblk * nb:BL + cblk * (nb + 1)],
                        start=(k == 0), stop=(k == KC - 1),
                    )
                evF = evFp.tile([T, cblk], F32)
                nc.vector.tensor_copy(evF, psF)
                nc.sync.dma_start(out=feats[:, cblk * nb:cblk * (nb + 1)], in_=evF)

    nc.compile()
    return nc


# ---------------------------------------------------------------------------
# host-side helpers
# ---------------------------------------------------------------------------

def _to_bf16_u16(a):
    """f32 ndarray -> uint16 bf16 bits, round-to-nearest-even (matches ml_dtypes)."""
    a = np.ascontiguousarray(a, np.float32)
    u = a.view(np.uint32)
    return ((u + np.uint32(0x7FFF) + ((u >> np.uint32(16)) & np.uint32(1)))
            >> np.uint32(16)).astype(np.uint16)


def _bf16(a):
    return _to_bf16_u16(a).view(BF16)


def _same(a, b):
    return a is b or (a.shape == b.shape and a.dtype == b.dtype
                      and np.array_equal(a, b))


def _make_runner(nc):
    import jax
    from jax.sharding import Mesh, PartitionSpec
    try:
        from jax.experimental.shard_map import shard_map
    except ImportError:
        from jax.shard_map import shard_map
    from concourse import bass2jax

    bass2jax.install_neuronx_cc_hook()

    partition_name = (nc.partition_id_tensor.name
                      if nc.partition_id_tensor is not None else None)

    in_names, out_names, out_avals, zero_outs = [], [], [], []
    for alloc in nc.m.functions[0].allocations:
        if not isinstance(alloc, mybir.MemoryLocationSet):
            continue
        name = alloc.memorylocations[0].name
        if alloc.kind == "ExternalInput":
            if name != partition_name:
                in_names.append(name)
        elif alloc.kind == "ExternalOutput":
            shape = tuple(alloc.tensor_shape)
            dtype = mybir.dt.np(alloc.dtype)
            out_names.append(name)
            out_avals.append(jax.core.ShapedArray(shape, dtype))
            zero_outs.append(np.zeros(shape, dtype))
    n_params = len(in_names)
    n_outs = len(out_avals)
    all_names = list(in_names) + list(out_names)
    if partition_name is not None:
        all_names.append(partition_name)

    donate = tuple(range(n_params, n_params + n_outs))

    def _body(*args):
        operands = list(args)
        if partition_name is not None:
            operands.append(bass2jax.partition_id_tensor())
        outs = bass2jax._bass_exec_p.bind(
            *operands,
            out_avals=tuple(out_avals),
            in_names=tuple(all_names),
            out_names=tuple(out_names),
            lowering_input_output_aliases=(),
            sim_require_finite=True,
            sim_require_nnan=True,
            nc=nc,
        )
        return tuple(outs)

    devices = jax.devices()[:8]
    mesh = Mesh(np.asarray(devices), ("core",))
    # emb is replicated (every core gathers from the full table); everything
    # else is concatenated per-core along axis 0.
    def spec_for(name):
        return PartitionSpec() if name in ("emb", "ident") else PartitionSpec("core")
    in_specs = tuple(spec_for(n) for n in in_names) + \
        tuple(PartitionSpec("core") for _ in range(n_outs))
    out_specs = tuple(PartitionSpec("core") for _ in out_names)
    sharded = jax.jit(
        shard_map(_body, mesh=mesh, in_specs=in_specs, out_specs=out_specs,
                  check_rep=False),
        donate_argnums=donate, keep_unused=True,
    )
    return {
        "fn": sharded,
        "mesh": mesh,
        "in_names": in_names,
        "out_names": out_names,
        "zero_shapes": [(tuple(a.shape), a.dtype) for a in out_avals],
    }


def _ensure_weights(runner, inputs):
    """Upload emb + per-direction weights if changed; return device arg dict."""
    import jax
    from jax.sharding import NamedSharding, PartitionSpec

    keys = ("emb", "w_ih_f", "w_hh_f", "b_f", "w_ih_b", "w_hh_b", "b_b", "w_out")
    cached = _state.get("weight_cache")
    if cached is not None and all(_same(inputs[k], cached["host"][k]) for k in keys):
        return cached["dev"]

    mesh = runner["mesh"]
    emb_bf = _bf16(inputs["emb"])                       # [V, E]

    def cat(parts_u16):
        return np.concatenate(parts_u16, axis=0)

    wih = {}
    whh = {}
    wout = {}
    bias = {}
    for d, (wi, wh, bb) in enumerate(
            [(inputs["w_ih_f"], inputs["w_hh_f"], inputs["b_f"]),
             (inputs["w_ih_b"], inputs["w_hh_b"], inputs["b_b"])]):
        wih[d] = _to_bf16_u16(np.ascontiguousarray(np.asarray(wi, np.float32).T))
        whh[d] = _to_bf16_u16(np.ascontiguousarray(np.asarray(wh, np.float32).T))
        bias[d] = np.ascontiguousarray(
            np.asarray(bb, np.float32).reshape(MC, 128).T)
    w_out = np.asarray(inputs["w_out"], np.float32)
    wout[0] = _to_bf16_u16(np.ascontiguousarray(w_out[:, :H].T))
    wout[1] = _to_bf16_u16(np.ascontiguousarray(w_out[:, H:].T))

    host_args = {
        "emb": emb_bf,
        "ident": np.eye(128, dtype=np.float32).astype(BF16),
        "w_ihT": cat([wih[0]] * 4 + [wih[1]] * 4).view(BF16),
        "w_hhT": cat([whh[0]] * 4 + [whh[1]] * 4).view(BF16),
        "bias_pm": np.concatenate([bias[0]] * 4 + [bias[1]] * 4, axis=0),
        "w_outT": cat([wout[0]] * 4 + [wout[1]] * 4).view(BF16),
    }
    dev = {}
    for name, arr in host_args.items():
        spec = PartitionSpec() if name in ("emb", "ident") else PartitionSpec("core")
        dev[name] = jax.device_put(arr, NamedSharding(mesh, spec))
    for a in dev.values():
        a.block_until_ready()
    _state["weight_cache"] = {
        "host": {k: inputs[k] for k in keys},
        "dev": dev,
    }
    return dev


def _logsumexp(a, axis):
    m = np.max(a, axis=axis, keepdims=True)
    return (m + np.log(np.sum(np.exp(a - m), axis=axis, keepdims=True))).squeeze(axis)


def _host_crf(feats, tags, mask, b_out, start_trans, end_trans, transitions):
    feats = feats + np.asarray(b_out, np.float32)[None, None, :]
    trans = np.asarray(transitions, np.float32)
    start = np.asarray(start_trans, np.float32)
    end = np.asarray(end_trans, np.float32)
    maskT = mask.T.astype(np.float32)       # [L, B]
    tagsT = tags.T                          # [L, B]
    em = np.take_along_axis(feats, tagsT[:, :, None], axis=2)[..., 0]  # [L, B]
    score = start[tagsT[0]] + em[0]
    tr = trans[tagsT[:-1], tagsT[1:]]
    score = score + ((tr + em[1:]) * maskT[1:]).sum(axis=0)
    last = mask.sum(axis=1).astype(np.int64) - 1
    last_tags = np.take_along_axis(tags, last[:, None], axis=1)[:, 0]
    score = score + end[last_tags]

    alpha = start[None, :] + feats[0]
    trb = trans[None, :, :]
    all_ones = bool(mask.all())
    for t in range(1, L):
        tmp = alpha[:, :, None] + trb
        tmp += feats[t][:, None, :]
        m = tmp.max(axis=1)
        np.exp(tmp - m[:, None, :], out=tmp)
        nxt = m + np.log(tmp.sum(axis=1))
        if all_ones:
            alpha = nxt
        else:
            alpha = np.where(maskT[t][:, None] > 0, nxt, alpha)
    denom = _logsumexp(alpha + end[None, :], axis=1)
    llh = score - denom
    return -(llh.sum() / maskT.sum())


_DEBUG_T = bool(os.environ.get("BASSK_TIME"))


def _tick(label, t0):
    if _DEBUG_T:
        t1 = time.perf_counter()
        print(f"[k] {label}: {(t1 - t0) * 1e3:.1f}ms", flush=True)
    return time.perf_counter()


def kernel(sentence, tags, mask, emb, w_ih_f, w_hh_f, b_f,
           w_ih_b, w_hh_b, b_b, w_out, b_out,
           start_trans, end_trans, transitions):
    t0 = time.perf_counter()
    sentence = np.asarray(sentence)
    tags = np.asarray(tags)
    mask = np.asarray(mask)

    if "nc" not in _state:
        _state["nc"] = _build_program()
        _state["runner"] = _make_runner(_state["nc"])
    runner = _state["runner"]

    t0 = _tick("setup", t0)
    dev = _ensure_weights(runner, {
        "emb": np.asarray(emb), "w_ih_f": np.asarray(w_ih_f),
        "w_hh_f": np.asarray(w_hh_f), "b_f": np.asarray(b_f),
        "w_ih_b": np.asarray(w_ih_b), "w_hh_b": np.asarray(w_hh_b),
        "b_b": np.asarray(b_b), "w_out": np.asarray(w_out),
    })

    t0 = _tick("weights", t0)
    # ---- per-call: token indices, t-major, reversed for backward cores ----
    # idx[p, j] = token id at t-major position j*128 + p
    idx_all = np.empty((8, 128, JB), np.int32)
    for c in range(8):
        sl = sentence[(c % 4) * BL:(c % 4) * BL + BL]   # [BL, L]
        tm = sl.T if c < 4 else sl.T[::-1]              # [L, BL] t-major
        flat = np.ascontiguousarray(tm).reshape(-1)
        idx_all[c] = flat.reshape(JB, 128).T
    idx_concat = idx_all.reshape(8 * 128, JB)

    t0 = _tick("idxprep", t0)
    args = {"idxs": idx_concat, **dev}
    ordered = [args[n] for n in runner["in_names"]]
    zeros = [np.zeros((8 * s[0], *s[1:]), d) for s, d in runner["zero_shapes"]]
    outs = runner["fn"](*ordered, *zeros)
    feats_cat = np.asarray(outs[runner["out_names"].index("feats")])
    t0 = _tick("jit+fetch", t0)

    # ---- assemble full feats [L, B, T] ----
    f = feats_cat.reshape(8, T, L, BL).transpose(0, 2, 3, 1)  # [8, L, BL, T]
    feats_full = np.empty((L, B, T), np.float32)
    for s in range(4):
        feats_full[:, s * BL:(s + 1) * BL, :] = f[s] + f[4 + s, ::-1]

    t0 = _tick("assemble", t0)
    loss = _host_crf(feats_full, tags, mask, b_out, start_trans, end_trans,
                     transitions)
    t0 = _tick("crf", t0)
    return np.float32(loss)


# revision 15
# speedup vs baseline: 1.2758x; 1.0058x over previous
"""BiLSTM-CRF loss kernel for 8 Trainium2 NeuronCores.

Sharding: direction x batch split. Cores 0-3 run the forward LSTM on batch
slices of 16 sequences; cores 4-7 run the backward LSTM (token order reversed
via the gather indices). The embedding table and all weights live device-side
(re-uploaded only if their values change); each warm call ships just the int32
token indices (~256 KB), runs one cached jit dispatch, and pulls back partial
emission features. Embedding gather runs on-device via gpsimd
indirect_dma_start (128 rows per DMA, one row index per partition), followed
by PE-array transposes into the [E, tokens] layout the input projection
needs. The tiny CRF runs on host.
"""

import os
import time
import numpy as np
import ml_dtypes

import concourse.bass as bass
import concourse.mybir as mybir
import concourse.tile as tile
from concourse import bacc

BF16 = ml_dtypes.bfloat16

B, L, V, E, HD, T = 64, 512, 32000, 512, 1024, 10
H = HD // 2          # 512 per-direction hidden
G4 = 4 * H           # 2048 gate rows
BL = 16              # sequences per core (64 batch / 4 slices; dirs split 0-3/4-7)
NC = L * BL          # 8192 (t-major columns: col = t*BL + b)
KC = H // 128        # 4 contraction chunks
MC = G4 // 128       # 16 gate-row chunks
NB = NC // 512       # 16 column blocks for the input projection
JB = NC // 128       # 64 gather blocks of 128 tokens
SB = 8               # recurrence steps per pre-gate prefetch block

F32 = mybir.dt.float32
BF16_T = mybir.dt.bfloat16
I32 = mybir.dt.int32
AF = mybir.ActivationFunctionType

_state = {}


def _build_program(steps=L):
    nc = bacc.Bacc("TRN2", target_bir_lowering=False, debug=False, num_devices=8)

    emb = nc.dram_tensor("emb", [V, E], BF16_T, kind="ExternalInput").ap()
    idxs = nc.dram_tensor("idxs", [128, JB], I32, kind="ExternalInput").ap()
    ident = nc.dram_tensor("ident", [128, 128], BF16_T, kind="ExternalInput").ap()
    w_ihT = nc.dram_tensor("w_ihT", [E, G4], BF16_T, kind="ExternalInput").ap()
    w_hhT = nc.dram_tensor("w_hhT", [H, G4], BF16_T, kind="ExternalInput").ap()
    bias_pm = nc.dram_tensor("bias_pm", [128, MC], F32, kind="ExternalInput").ap()
    w_outT = nc.dram_tensor("w_outT", [H, T], BF16_T, kind="ExternalInput").ap()
    feats = nc.dram_tensor("feats", [T, NC], BF16_T, kind="ExternalOutput").ap()
    pre = nc.dram_tensor("pre", [MC, 128, NC], F32).ap()  # scratch in DRAM

    with tile.TileContext(nc) as tc:
        with (
            tc.tile_pool(name="singles", bufs=1) as singles,
            tc.tile_pool(name="xin", bufs=1) as xin,
            tc.tile_pool(name="xn", bufs=4) as xnp,
            tc.tile_pool(name="psT", bufs=2, space="PSUM") as psTp,
            tc.tile_pool(name="psA", bufs=2, space="PSUM") as psA,
            tc.tile_pool(name="evA", bufs=4) as evA,
            tc.tile_pool(name="prestream", bufs=2) as prestream,
            tc.tile_pool(name="psB", bufs=2, space="PSUM") as psB,
            tc.tile_pool(name="gtmp", bufs=2) as gtmp,
            tc.tile_pool(name="atmp", bufs=2) as atmp,
            tc.tile_pool(name="stmp", bufs=3) as stmp,
            tc.tile_pool(name="psF", bufs=2, space="PSUM") as psFp,
            tc.tile_pool(name="evF", bufs=2) as evFp,
        ):
            # ---- resident weights ----
            wih_sb = [singles.tile([128, G4], BF16_T, tag=f"wih{k}", name=f"wih{k}") for k in range(KC)]
            whh_sb = [singles.tile([128, G4], BF16_T, tag=f"whh{k}", name=f"whh{k}") for k in range(KC)]
            for k in range(KC):
                nc.sync.dma_start(out=wih_sb[k], in_=w_ihT[128 * k:128 * (k + 1), :])
                nc.sync.dma_start(out=whh_sb[k], in_=w_hhT[128 * k:128 * (k + 1), :])
            bias_sb = singles.tile([128, MC], F32, tag="bias")
            nc.sync.dma_start(out=bias_sb, in_=bias_pm)
            wout_sb = [singles.tile([128, T], BF16_T, tag=f"wo{k}", name=f"wo{k}") for k in range(KC)]
            for k in range(KC):
                nc.sync.dma_start(out=wout_sb[k], in_=w_outT[128 * k:128 * (k + 1), :])

            # ---- on-device embedding gather + PE transpose to [E, tokens] ----
            idx_sb = singles.tile([128, JB], I32, tag="idx")
            nc.sync.dma_start(out=idx_sb, in_=idxs)
            ident_sb = singles.tile([128, 128], BF16_T, tag="ident")
            nc.sync.dma_start(out=ident_sb, in_=ident)

            xT = xin.tile([128, KC, NC], BF16_T, tag="xT")
            for j in range(JB):
                xn = xnp.tile([128, E], BF16_T)
                nc.gpsimd.indirect_dma_start(
                    out=xn, out_offset=None, in_=emb,
                    in_offset=bass.IndirectOffsetOnAxis(
                        ap=idx_sb[:, j:j + 1], axis=0),
                )
                for k in range(KC):
                    psT = psTp.tile([128, 128], BF16_T)
                    nc.tensor.transpose(psT, xn[:, 128 * k:128 * (k + 1)], ident_sb)
                    nc.vector.tensor_copy(xT[:, k, 128 * j:128 * (j + 1)], psT)

            # ---- phase A: pre-gates = W_ih @ x (+bias), streamed to DRAM ----
            for m in range(MC):
                for nb in range(NB):
                    ps = psA.tile([128, 512], F32)
                    for k in range(KC):
                        nc.tensor.matmul(
                            ps,
                            wih_sb[k][:, 128 * m:128 * (m + 1)],
                            xT[:, k, 512 * nb:512 * (nb + 1)],
                            start=(k == 0), stop=(k == KC - 1),
                        )
                    ev = evA.tile([128, 512], F32)
                    nc.scalar.activation(ev, ps, AF.Identity,
                                         bias=bias_sb[:, m:m + 1])
                    nc.sync.dma_start(out=pre[m, :, 512 * nb:512 * (nb + 1)], in_=ev)

            # ---- phase B: recurrence ----
            # h history: [128, KC, (steps+1)*BL] bf16; col block s holds h_{s-1}
            hh = singles.tile([128, KC, (steps + 1) * BL], BF16_T, tag="hh")
            nc.vector.memset(hh[:, :, 0:BL], 0.0)
            c_sb = singles.tile([128, KC, BL], F32, tag="c")
            nc.vector.memset(c_sb, 0.0)

            preT = pre.rearrange("m p c -> p m c")
            for tb in range(steps // SB):
                pt = prestream.tile([128, MC, SB * BL], F32)
                nc.sync.dma_start(
                    out=pt, in_=preT[:, :, BL * SB * tb:BL * SB * (tb + 1)])
                for ts in range(SB):
                    t = tb * SB + ts
                    ptt = pt[:, :, BL * ts:BL * (ts + 1)]     # [128, MC, BL]
                    ps = psB.tile([128, MC, BL], F32)
                    hprev = hh[:, :, BL * t:BL * (t + 1)]     # [128, KC, BL]
                    for m in range(MC):
                        for k in range(KC):
                            nc.tensor.matmul(
                                ps[:, m, :],
                                whh_sb[k][:, 128 * m:128 * (m + 1)],
                                hprev[:, k, :],
                                start=(k == 0), stop=(k == KC - 1),
                            )
                    g_sb = gtmp.tile([128, MC, BL], F32)
                    # i,f block ready after m=7; g,o after m=15
                    nc.vector.tensor_add(g_sb[:, 0:8, :], ps[:, 0:8, :], ptt[:, 0:8, :])
                    nc.vector.tensor_add(g_sb[:, 8:16, :], ps[:, 8:16, :], ptt[:, 8:16, :])
                    a_sb = atmp.tile([128, MC, BL], F32)
                    nc.scalar.activation(a_sb[:, 0:8, :], g_sb[:, 0:8, :], AF.Sigmoid)
                    nc.scalar.activation(a_sb[:, 8:12, :], g_sb[:, 8:12, :], AF.Tanh)
                    nc.scalar.activation(a_sb[:, 12:16, :], g_sb[:, 12:16, :], AF.Sigmoid)
                    t1 = stmp.tile([128, KC, BL], F32, tag="t1")
                    nc.vector.tensor_mul(t1, a_sb[:, 0:4, :], a_sb[:, 8:12, :])
                    nc.vector.tensor_mul(c_sb, a_sb[:, 4:8, :], c_sb)
                    nc.vector.tensor_add(c_sb, c_sb, t1)
                    tcn = stmp.tile([128, KC, BL], F32, tag="tc")
                    nc.scalar.activation(tcn, c_sb, AF.Tanh)
                    hout = hh[:, :, BL * (t + 1):BL * (t + 2)]
                    nc.vector.tensor_mul(hout, a_sb[:, 12:16, :], tcn)

            # ---- phase C: partial feats = w_out_half.T @ h ----
            ncols_h = steps * BL
            cblk = min(512, ncols_h)
            for nb in range(ncols_h // cblk):
                psF = psFp.tile([T, cblk], F32)
                for k in range(KC):
                    nc.tensor.matmul(
                        psF,
                        wout_sb[k],
                        hh[:, k, BL + cblk * nb:BL + cblk * (nb + 1)],
                        start=(k == 0), stop=(k == KC - 1),
                    )
                evF = evFp.tile([T, cblk], BF16_T)
                nc.vector.tensor_copy(evF, psF)
                nc.sync.dma_start(out=feats[:, cblk * nb:cblk * (nb + 1)], in_=evF)

    nc.compile()
    return nc


# ---------------------------------------------------------------------------
# host-side helpers
# ---------------------------------------------------------------------------

def _to_bf16_u16(a):
    """f32 ndarray -> uint16 bf16 bits, round-to-nearest-even (matches ml_dtypes)."""
    a = np.ascontiguousarray(a, np.float32)
    u = a.view(np.uint32)
    return ((u + np.uint32(0x7FFF) + ((u >> np.uint32(16)) & np.uint32(1)))
            >> np.uint32(16)).astype(np.uint16)


def _bf16(a):
    return _to_bf16_u16(a).view(BF16)


def _same(a, b):
    return a is b or (a.shape == b.shape and a.dtype == b.dtype
                      and np.array_equal(a, b))


def _make_runner(nc):
    import jax
    from jax.sharding import Mesh, PartitionSpec
    try:
        from jax.experimental.shard_map import shard_map
    except ImportError:
        from jax.shard_map import shard_map
    from concourse import bass2jax

    bass2jax.install_neuronx_cc_hook()

    partition_name = (nc.partition_id_tensor.name
                      if nc.partition_id_tensor is not None else None)

    in_names, out_names, out_avals, zero_outs = [], [], [], []
    for alloc in nc.m.functions[0].allocations:
        if not isinstance(alloc, mybir.MemoryLocationSet):
            continue
        name = alloc.memorylocations[0].name
        if alloc.kind == "ExternalInput":
            if name != partition_name:
                in_names.append(name)
        elif alloc.kind == "ExternalOutput":
            shape = tuple(alloc.tensor_shape)
            dtype = mybir.dt.np(alloc.dtype)
            out_names.append(name)
            out_avals.append(jax.core.ShapedArray(shape, dtype))
            zero_outs.append(np.zeros(shape, dtype))
    n_params = len(in_names)
    n_outs = len(out_avals)
    all_names = list(in_names) + list(out_names)
    if partition_name is not None:
        all_names.append(partition_name)

    donate = tuple(range(n_params, n_params + n_outs))

    def _body(*args):
        operands = list(args)
        if partition_name is not None:
            operands.append(bass2jax.partition_id_tensor())
        outs = bass2jax._bass_exec_p.bind(
            *operands,
            out_avals=tuple(out_avals),
            in_names=tuple(all_names),
            out_names=tuple(out_names),
            lowering_input_output_aliases=(),
            sim_require_finite=True,
            sim_require_nnan=True,
            nc=nc,
        )
        return tuple(outs)

    devices = jax.devices()[:8]
    mesh = Mesh(np.asarray(devices), ("core",))
    # emb is replicated (every core gathers from the full table); everything
    # else is concatenated per-core along axis 0.
    def spec_for(name):
        return PartitionSpec() if name in ("emb", "ident") else PartitionSpec("core")
    in_specs = tuple(spec_for(n) for n in in_names) + \
        tuple(PartitionSpec("core") for _ in range(n_outs))
    out_specs = tuple(PartitionSpec("core") for _ in out_names)
    sharded = jax.jit(
        shard_map(_body, mesh=mesh, in_specs=in_specs, out_specs=out_specs,
                  check_rep=False),
        donate_argnums=donate, keep_unused=True,
    )
    return {
        "fn": sharded,
        "mesh": mesh,
        "in_names": in_names,
        "out_names": out_names,
        "zero_shapes": [(tuple(a.shape), a.dtype) for a in out_avals],
    }


def _ensure_weights(runner, inputs):
    """Upload emb + per-direction weights if changed; return device arg dict."""
    import jax
    from jax.sharding import NamedSharding, PartitionSpec

    keys = ("emb", "w_ih_f", "w_hh_f", "b_f", "w_ih_b", "w_hh_b", "b_b", "w_out")
    cached = _state.get("weight_cache")
    if cached is not None and all(_same(inputs[k], cached["host"][k]) for k in keys):
        return cached["dev"]

    mesh = runner["mesh"]
    emb_bf = _bf16(inputs["emb"])                       # [V, E]

    def cat(parts_u16):
        return np.concatenate(parts_u16, axis=0)

    wih = {}
    whh = {}
    wout = {}
    bias = {}
    for d, (wi, wh, bb) in enumerate(
            [(inputs["w_ih_f"], inputs["w_hh_f"], inputs["b_f"]),
             (inputs["w_ih_b"], inputs["w_hh_b"], inputs["b_b"])]):
        wih[d] = _to_bf16_u16(np.ascontiguousarray(np.asarray(wi, np.float32).T))
        whh[d] = _to_bf16_u16(np.ascontiguousarray(np.asarray(wh, np.float32).T))
        bias[d] = np.ascontiguousarray(
            np.asarray(bb, np.float32).reshape(MC, 128).T)
    w_out = np.asarray(inputs["w_out"], np.float32)
    wout[0] = _to_bf16_u16(np.ascontiguousarray(w_out[:, :H].T))
    wout[1] = _to_bf16_u16(np.ascontiguousarray(w_out[:, H:].T))

    host_args = {
        "emb": emb_bf,
        "ident": np.eye(128, dtype=np.float32).astype(BF16),
        "w_ihT": cat([wih[0]] * 4 + [wih[1]] * 4).view(BF16),
        "w_hhT": cat([whh[0]] * 4 + [whh[1]] * 4).view(BF16),
        "bias_pm": np.concatenate([bias[0]] * 4 + [bias[1]] * 4, axis=0),
        "w_outT": cat([wout[0]] * 4 + [wout[1]] * 4).view(BF16),
    }
    dev = {}
    for name, arr in host_args.items():
        spec = PartitionSpec() if name in ("emb", "ident") else PartitionSpec("core")
        dev[name] = jax.device_put(arr, NamedSharding(mesh, spec))
    for a in dev.values():
        a.block_until_ready()
    _state["weight_cache"] = {
        "host": {k: inputs[k] for k in keys},
        "dev": dev,
    }
    return dev


try:
    import numba

    @numba.njit(cache=False, fastmath=False)
    def _crf_denom_nb(feats, expT, start, end, maskT):
        # feats [L, B, T] f32, expT [T, T] f64; returns denom [B] f64
        Ln, Bn, Tn = feats.shape
        denom = np.empty(Bn, np.float64)
        alpha = np.empty(Tn, np.float64)
        ea = np.empty(Tn, np.float64)
        for b in range(Bn):
            for j in range(Tn):
                alpha[j] = start[j] + feats[0, b, j]
            for t in range(1, Ln):
                if maskT[t, b] > 0.0:
                    m = alpha[0]
                    for i in range(1, Tn):
                        if alpha[i] > m:
                            m = alpha[i]
                    for i in range(Tn):
                        ea[i] = np.exp(alpha[i] - m)
                    for j in range(Tn):
                        s = 0.0
                        for i in range(Tn):
                            s += ea[i] * expT[i, j]
                        alpha[j] = m + np.log(s) + feats[t, b, j]
            m = alpha[0] + end[0]
            for j in range(1, Tn):
                if alpha[j] + end[j] > m:
                    m = alpha[j] + end[j]
            s = 0.0
            for j in range(Tn):
                s += np.exp(alpha[j] + end[j] - m)
            denom[b] = m + np.log(s)
        return denom

    _HAVE_NUMBA = True
except Exception:
    _HAVE_NUMBA = False


def _logsumexp(a, axis):
    m = np.max(a, axis=axis, keepdims=True)
    return (m + np.log(np.sum(np.exp(a - m), axis=axis, keepdims=True))).squeeze(axis)


def _host_crf(feats, tags, mask, b_out, start_trans, end_trans, transitions):
    feats = feats + np.asarray(b_out, np.float32)[None, None, :]
    trans = np.asarray(transitions, np.float32)
    start = np.asarray(start_trans, np.float32)
    end = np.asarray(end_trans, np.float32)
    maskT = mask.T.astype(np.float32)       # [L, B]
    tagsT = tags.T                          # [L, B]
    em = np.take_along_axis(feats, tagsT[:, :, None], axis=2)[..., 0]  # [L, B]
    score = start[tagsT[0]] + em[0]
    tr = trans[tagsT[:-1], tagsT[1:]]
    score = score + ((tr + em[1:]) * maskT[1:]).sum(axis=0)
    last = mask.sum(axis=1).astype(np.int64) - 1
    last_tags = np.take_along_axis(tags, last[:, None], axis=1)[:, 0]
    score = score + end[last_tags]

    if _HAVE_NUMBA:
        denom = _crf_denom_nb(
            np.ascontiguousarray(feats, np.float32),
            np.exp(np.asarray(transitions, np.float64)),
            start.astype(np.float64), end.astype(np.float64),
            np.ascontiguousarray(maskT, np.float64))
    else:
        alpha = start[None, :] + feats[0]
        trb = trans[None, :, :]
        for t in range(1, L):
            tmp = alpha[:, :, None] + trb
            tmp += feats[t][:, None, :]
            m = tmp.max(axis=1)
            np.exp(tmp - m[:, None, :], out=tmp)
            nxt = m + np.log(tmp.sum(axis=1))
            alpha = np.where(maskT[t][:, None] > 0, nxt, alpha)
        denom = _logsumexp(alpha + end[None, :], axis=1)
    llh = score - denom
    return -(llh.sum() / maskT.sum())


_DEBUG_T = bool(os.environ.get("BASSK_TIME"))


def _tick(label, t0):
    if _DEBUG_T:
        t1 = time.perf_counter()
        print(f"[k] {label}: {(t1 - t0) * 1e3:.1f}ms", flush=True)
    return time.perf_counter()


def kernel(sentence, tags, mask, emb, w_ih_f, w_hh_f, b_f,
           w_ih_b, w_hh_b, b_b, w_out, b_out,
           start_trans, end_trans, transitions):
    t0 = time.perf_counter()
    sentence = np.asarray(sentence)
    tags = np.asarray(tags)
    mask = np.asarray(mask)

    if "nc" not in _state:
        _state["nc"] = _build_program()
        _state["runner"] = _make_runner(_state["nc"])
    runner = _state["runner"]

    t0 = _tick("setup", t0)
    dev = _ensure_weights(runner, {
        "emb": np.asarray(emb), "w_ih_f": np.asarray(w_ih_f),
        "w_hh_f": np.asarray(w_hh_f), "b_f": np.asarray(b_f),
        "w_ih_b": np.asarray(w_ih_b), "w_hh_b": np.asarray(w_hh_b),
        "b_b": np.asarray(b_b), "w_out": np.asarray(w_out),
    })

    t0 = _tick("weights", t0)
    # ---- per-call: token indices, t-major, reversed for backward cores ----
    # idx[p, j] = token id at t-major position j*128 + p
    idx_all = np.empty((8, 128, JB), np.int32)
    for c in range(8):
        sl = sentence[(c % 4) * BL:(c % 4) * BL + BL]   # [BL, L]
        tm = sl.T if c < 4 else sl.T[::-1]              # [L, BL] t-major
        flat = np.ascontiguousarray(tm).reshape(-1)
        idx_all[c] = flat.reshape(JB, 128).T
    idx_concat = idx_all.reshape(8 * 128, JB)

    t0 = _tick("idxprep", t0)
    args = {"idxs": idx_concat, **dev}
    ordered = [args[n] for n in runner["in_names"]]
    zeros = [np.zeros((8 * s[0], *s[1:]), d) for s, d in runner["zero_shapes"]]
    outs = runner["fn"](*ordered, *zeros)
    feats_cat = np.asarray(outs[runner["out_names"].index("feats")])
    feats_cat = (feats_cat.view(np.uint16).astype(np.uint32) << np.uint32(16)).view(np.float32)
    t0 = _tick("jit+fetch", t0)

    # ---- assemble full feats [L, B, T] ----
    f = feats_cat.reshape(8, T, L, BL).transpose(0, 2, 3, 1)  # [8, L, BL, T]
    feats_full = np.empty((L, B, T), np.float32)
    for s in range(4):
        feats_full[:, s * BL:(s + 1) * BL, :] = f[s] + f[4 + s, ::-1]

    t0 = _tick("assemble", t0)
    loss = _host_crf(feats_full, tags, mask, b_out, start_trans, end_trans,
                     transitions)
    t0 = _tick("crf", t0)
    return np.float32(loss)


# revision 16
# speedup vs baseline: 1.9277x; 1.5110x over previous
"""BiLSTM-CRF loss kernel for 8 Trainium2 NeuronCores.

Sharding: direction x batch split. Cores 0-3 run the forward LSTM on batch
slices of 16 sequences; cores 4-7 run the backward LSTM (token order reversed
via the gather indices). The embedding table and all weights live device-side
(re-uploaded only if their values change); each warm call ships just the int32
token indices (~256 KB), runs one cached jit dispatch, and pulls back partial
emission features. Embedding gather runs on-device via gpsimd
indirect_dma_start (128 rows per DMA, one row index per partition), followed
by PE-array transposes into the [E, tokens] layout the input projection
needs. The tiny CRF runs on host.
"""

import os
import time
import numpy as np
import ml_dtypes

import concourse.bass as bass
import concourse.mybir as mybir
import concourse.tile as tile
from concourse import bacc

BF16 = ml_dtypes.bfloat16

B, L, V, E, HD, T = 64, 512, 32000, 512, 1024, 10
H = HD // 2          # 512 per-direction hidden
G4 = 4 * H           # 2048 gate rows
BL = 16              # sequences per core (64 batch / 4 slices; dirs split 0-3/4-7)
NC = L * BL          # 8192 (t-major columns: col = t*BL + b)
KC = H // 128        # 4 contraction chunks
MC = G4 // 128       # 16 gate-row chunks
NB = NC // 512       # 16 column blocks for the input projection
JB = NC // 128       # 64 gather blocks of 128 tokens
SB = 8               # recurrence steps per pre-gate prefetch block

F32 = mybir.dt.float32
BF16_T = mybir.dt.bfloat16
I32 = mybir.dt.int32
AF = mybir.ActivationFunctionType

_state = {}


def _build_program(steps=L):
    nc = bacc.Bacc("TRN2", target_bir_lowering=False, debug=False, num_devices=8)

    emb = nc.dram_tensor("emb", [V, E], BF16_T, kind="ExternalInput").ap()
    idxs = nc.dram_tensor("idxs", [128, JB], I32, kind="ExternalInput").ap()
    ident = nc.dram_tensor("ident", [128, 128], BF16_T, kind="ExternalInput").ap()
    w_ihT = nc.dram_tensor("w_ihT", [E, G4], BF16_T, kind="ExternalInput").ap()
    w_hhT = nc.dram_tensor("w_hhT", [H, G4], BF16_T, kind="ExternalInput").ap()
    bias_pm = nc.dram_tensor("bias_pm", [128, MC], F32, kind="ExternalInput").ap()
    w_outT = nc.dram_tensor("w_outT", [H, T], BF16_T, kind="ExternalInput").ap()
    feats = nc.dram_tensor("feats", [T, NC], BF16_T, kind="ExternalOutput").ap()
    pre = nc.dram_tensor("pre", [MC, 128, NC], F32).ap()  # scratch in DRAM

    with tile.TileContext(nc) as tc:
        with (
            tc.tile_pool(name="singles", bufs=1) as singles,
            tc.tile_pool(name="xin", bufs=1) as xin,
            tc.tile_pool(name="xn", bufs=4) as xnp,
            tc.tile_pool(name="psT", bufs=2, space="PSUM") as psTp,
            tc.tile_pool(name="psA", bufs=2, space="PSUM") as psA,
            tc.tile_pool(name="evA", bufs=4) as evA,
            tc.tile_pool(name="prestream", bufs=2) as prestream,
            tc.tile_pool(name="psB", bufs=2, space="PSUM") as psB,
            tc.tile_pool(name="gtmp", bufs=2) as gtmp,
            tc.tile_pool(name="atmp", bufs=2) as atmp,
            tc.tile_pool(name="stmp", bufs=3) as stmp,
            tc.tile_pool(name="psF", bufs=2, space="PSUM") as psFp,
            tc.tile_pool(name="evF", bufs=2) as evFp,
        ):
            # ---- resident weights ----
            wih_sb = [singles.tile([128, G4], BF16_T, tag=f"wih{k}", name=f"wih{k}") for k in range(KC)]
            whh_sb = [singles.tile([128, G4], BF16_T, tag=f"whh{k}", name=f"whh{k}") for k in range(KC)]
            for k in range(KC):
                nc.sync.dma_start(out=wih_sb[k], in_=w_ihT[128 * k:128 * (k + 1), :])
                nc.sync.dma_start(out=whh_sb[k], in_=w_hhT[128 * k:128 * (k + 1), :])
            bias_sb = singles.tile([128, MC], F32, tag="bias")
            nc.sync.dma_start(out=bias_sb, in_=bias_pm)
            wout_sb = [singles.tile([128, T], BF16_T, tag=f"wo{k}", name=f"wo{k}") for k in range(KC)]
            for k in range(KC):
                nc.sync.dma_start(out=wout_sb[k], in_=w_outT[128 * k:128 * (k + 1), :])

            # ---- on-device embedding gather + PE transpose to [E, tokens] ----
            idx_sb = singles.tile([128, JB], I32, tag="idx")
            nc.sync.dma_start(out=idx_sb, in_=idxs)
            ident_sb = singles.tile([128, 128], BF16_T, tag="ident")
            nc.sync.dma_start(out=ident_sb, in_=ident)

            xT = xin.tile([128, KC, NC], BF16_T, tag="xT")
            for j in range(JB):
                xn = xnp.tile([128, E], BF16_T)
                nc.gpsimd.indirect_dma_start(
                    out=xn, out_offset=None, in_=emb,
                    in_offset=bass.IndirectOffsetOnAxis(
                        ap=idx_sb[:, j:j + 1], axis=0),
                )
                for k in range(KC):
                    psT = psTp.tile([128, 128], BF16_T)
                    nc.tensor.transpose(psT, xn[:, 128 * k:128 * (k + 1)], ident_sb)
                    nc.vector.tensor_copy(xT[:, k, 128 * j:128 * (j + 1)], psT)

            # ---- phase A: pre-gates = W_ih @ x (+bias), streamed to DRAM ----
            for m in range(MC):
                for nb in range(NB):
                    ps = psA.tile([128, 512], F32)
                    for k in range(KC):
                        nc.tensor.matmul(
                            ps,
                            wih_sb[k][:, 128 * m:128 * (m + 1)],
                            xT[:, k, 512 * nb:512 * (nb + 1)],
                            start=(k == 0), stop=(k == KC - 1),
                        )
                    ev = evA.tile([128, 512], F32)
                    nc.scalar.activation(ev, ps, AF.Identity,
                                         bias=bias_sb[:, m:m + 1])
                    nc.sync.dma_start(out=pre[m, :, 512 * nb:512 * (nb + 1)], in_=ev)

            # ---- phase B: recurrence ----
            # h history: [128, KC, (steps+1)*BL] bf16; col block s holds h_{s-1}
            hh = singles.tile([128, KC, (steps + 1) * BL], BF16_T, tag="hh")
            nc.vector.memset(hh[:, :, 0:BL], 0.0)
            c_sb = singles.tile([128, KC, BL], F32, tag="c")
            nc.vector.memset(c_sb, 0.0)

            preT = pre.rearrange("m p c -> p m c")
            for tb in range(steps // SB):
                pt = prestream.tile([128, MC, SB * BL], F32)
                nc.sync.dma_start(
                    out=pt, in_=preT[:, :, BL * SB * tb:BL * SB * (tb + 1)])
                for ts in range(SB):
                    t = tb * SB + ts
                    ptt = pt[:, :, BL * ts:BL * (ts + 1)]     # [128, MC, BL]
                    ps = psB.tile([128, MC, BL], F32)
                    hprev = hh[:, :, BL * t:BL * (t + 1)]     # [128, KC, BL]
                    for m in range(MC):
                        for k in range(KC):
                            nc.tensor.matmul(
                                ps[:, m, :],
                                whh_sb[k][:, 128 * m:128 * (m + 1)],
                                hprev[:, k, :],
                                start=(k == 0), stop=(k == KC - 1),
                            )
                    g_sb = gtmp.tile([128, MC, BL], F32)
                    # i,f block ready after m=7; g,o after m=15
                    nc.vector.tensor_add(g_sb[:, 0:8, :], ps[:, 0:8, :], ptt[:, 0:8, :])
                    nc.vector.tensor_add(g_sb[:, 8:16, :], ps[:, 8:16, :], ptt[:, 8:16, :])
                    a_sb = atmp.tile([128, MC, BL], F32)
                    nc.scalar.activation(a_sb[:, 0:8, :], g_sb[:, 0:8, :], AF.Sigmoid)
                    nc.scalar.activation(a_sb[:, 8:12, :], g_sb[:, 8:12, :], AF.Tanh)
                    nc.scalar.activation(a_sb[:, 12:16, :], g_sb[:, 12:16, :], AF.Sigmoid)
                    t1 = stmp.tile([128, KC, BL], F32, tag="t1")
                    nc.vector.tensor_mul(t1, a_sb[:, 0:4, :], a_sb[:, 8:12, :])
                    nc.vector.tensor_mul(c_sb, a_sb[:, 4:8, :], c_sb)
                    nc.vector.tensor_add(c_sb, c_sb, t1)
                    tcn = stmp.tile([128, KC, BL], F32, tag="tc")
                    nc.scalar.activation(tcn, c_sb, AF.Tanh)
                    hout = hh[:, :, BL * (t + 1):BL * (t + 2)]
                    nc.vector.tensor_mul(hout, a_sb[:, 12:16, :], tcn)

            # ---- phase C: partial feats = w_out_half.T @ h ----
            ncols_h = steps * BL
            cblk = min(512, ncols_h)
            for nb in range(ncols_h // cblk):
                psF = psFp.tile([T, cblk], F32)
                for k in range(KC):
                    nc.tensor.matmul(
                        psF,
                        wout_sb[k],
                        hh[:, k, BL + cblk * nb:BL + cblk * (nb + 1)],
                        start=(k == 0), stop=(k == KC - 1),
                    )
                evF = evFp.tile([T, cblk], BF16_T)
                nc.vector.tensor_copy(evF, psF)
                nc.sync.dma_start(out=feats[:, cblk * nb:cblk * (nb + 1)], in_=evF)

    nc.compile()
    return nc


# ---------------------------------------------------------------------------
# host-side helpers
# ---------------------------------------------------------------------------

def _to_bf16_u16(a):
    """f32 ndarray -> uint16 bf16 bits, round-to-nearest-even (matches ml_dtypes)."""
    a = np.ascontiguousarray(a, np.float32)
    u = a.view(np.uint32)
    return ((u + np.uint32(0x7FFF) + ((u >> np.uint32(16)) & np.uint32(1)))
            >> np.uint32(16)).astype(np.uint16)


def _bf16(a):
    return _to_bf16_u16(a).view(BF16)


def _same(a, b):
    return a is b or (a.shape == b.shape and a.dtype == b.dtype
                      and np.array_equal(a, b))


def _make_runner(nc):
    import jax
    from jax.sharding import Mesh, PartitionSpec
    try:
        from jax.experimental.shard_map import shard_map
    except ImportError:
        from jax.shard_map import shard_map
    from concourse import bass2jax

    bass2jax.install_neuronx_cc_hook()

    partition_name = (nc.partition_id_tensor.name
                      if nc.partition_id_tensor is not None else None)

    in_names, out_names, out_avals, zero_outs = [], [], [], []
    for alloc in nc.m.functions[0].allocations:
        if not isinstance(alloc, mybir.MemoryLocationSet):
            continue
        name = alloc.memorylocations[0].name
        if alloc.kind == "ExternalInput":
            if name != partition_name:
                in_names.append(name)
        elif alloc.kind == "ExternalOutput":
            shape = tuple(alloc.tensor_shape)
            dtype = mybir.dt.np(alloc.dtype)
            out_names.append(name)
            out_avals.append(jax.core.ShapedArray(shape, dtype))
            zero_outs.append(np.zeros(shape, dtype))
    n_params = len(in_names)
    n_outs = len(out_avals)
    all_names = list(in_names) + list(out_names)
    if partition_name is not None:
        all_names.append(partition_name)

    donate = tuple(range(n_params, n_params + n_outs))

    def _body(*args):
        operands = list(args)
        if partition_name is not None:
            operands.append(bass2jax.partition_id_tensor())
        outs = bass2jax._bass_exec_p.bind(
            *operands,
            out_avals=tuple(out_avals),
            in_names=tuple(all_names),
            out_names=tuple(out_names),
            lowering_input_output_aliases=(),
            sim_require_finite=True,
            sim_require_nnan=True,
            nc=nc,
        )
        return tuple(outs)

    devices = jax.devices()[:8]
    mesh = Mesh(np.asarray(devices), ("core",))
    # emb is replicated (every core gathers from the full table); everything
    # else is concatenated per-core along axis 0.
    def spec_for(name):
        return PartitionSpec() if name in ("emb", "ident") else PartitionSpec("core")
    in_specs = tuple(spec_for(n) for n in in_names) + \
        tuple(PartitionSpec("core") for _ in range(n_outs))
    out_specs = tuple(PartitionSpec("core") for _ in out_names)
    sharded = jax.jit(
        shard_map(_body, mesh=mesh, in_specs=in_specs, out_specs=out_specs,
                  check_rep=False),
        donate_argnums=donate, keep_unused=True,
    )
    return {
        "fn": sharded,
        "mesh": mesh,
        "in_names": in_names,
        "out_names": out_names,
        "zero_shapes": [(tuple(a.shape), a.dtype) for a in out_avals],
    }


def _ensure_weights(runner, inputs):
    """Upload emb + per-direction weights if changed; return device arg dict."""
    import jax
    from jax.sharding import NamedSharding, PartitionSpec

    keys = ("emb", "w_ih_f", "w_hh_f", "b_f", "w_ih_b", "w_hh_b", "b_b", "w_out")
    cached = _state.get("weight_cache")
    if cached is not None and all(_same(inputs[k], cached["host"][k]) for k in keys):
        return cached["dev"]

    mesh = runner["mesh"]
    emb_bf = _bf16(inputs["emb"])                       # [V, E]

    def cat(parts_u16):
        return np.concatenate(parts_u16, axis=0)

    wih = {}
    whh = {}
    wout = {}
    bias = {}
    for d, (wi, wh, bb) in enumerate(
            [(inputs["w_ih_f"], inputs["w_hh_f"], inputs["b_f"]),
             (inputs["w_ih_b"], inputs["w_hh_b"], inputs["b_b"])]):
        wih[d] = _to_bf16_u16(np.ascontiguousarray(np.asarray(wi, np.float32).T))
        whh[d] = _to_bf16_u16(np.ascontiguousarray(np.asarray(wh, np.float32).T))
        bias[d] = np.ascontiguousarray(
            np.asarray(bb, np.float32).reshape(MC, 128).T)
    w_out = np.asarray(inputs["w_out"], np.float32)
    wout[0] = _to_bf16_u16(np.ascontiguousarray(w_out[:, :H].T))
    wout[1] = _to_bf16_u16(np.ascontiguousarray(w_out[:, H:].T))

    host_args = {
        "emb": emb_bf,
        "ident": np.eye(128, dtype=np.float32).astype(BF16),
        "w_ihT": cat([wih[0]] * 4 + [wih[1]] * 4).view(BF16),
        "w_hhT": cat([whh[0]] * 4 + [whh[1]] * 4).view(BF16),
        "bias_pm": np.concatenate([bias[0]] * 4 + [bias[1]] * 4, axis=0),
        "w_outT": cat([wout[0]] * 4 + [wout[1]] * 4).view(BF16),
    }
    dev = {}
    for name, arr in host_args.items():
        spec = PartitionSpec() if name in ("emb", "ident") else PartitionSpec("core")
        dev[name] = jax.device_put(arr, NamedSharding(mesh, spec))
    for a in dev.values():
        a.block_until_ready()
    _state["weight_cache"] = {
        "host": {k: inputs[k] for k in keys},
        "dev": dev,
    }
    return dev


try:
    import numba

    @numba.njit(cache=False, fastmath=False)
    def _crf_denom_nb(feats, expT, start, end, maskT):
        # feats [L, B, T] f32, expT [T, T] f64; returns denom [B] f64
        Ln, Bn, Tn = feats.shape
        denom = np.empty(Bn, np.float64)
        alpha = np.empty(Tn, np.float64)
        ea = np.empty(Tn, np.float64)
        for b in range(Bn):
            for j in range(Tn):
                alpha[j] = start[j] + feats[0, b, j]
            for t in range(1, Ln):
                if maskT[t, b] > 0.0:
                    m = alpha[0]
                    for i in range(1, Tn):
                        if alpha[i] > m:
                            m = alpha[i]
                    for i in range(Tn):
                        ea[i] = np.exp(alpha[i] - m)
                    for j in range(Tn):
                        s = 0.0
                        for i in range(Tn):
                            s += ea[i] * expT[i, j]
                        alpha[j] = m + np.log(s) + feats[t, b, j]
            m = alpha[0] + end[0]
            for j in range(1, Tn):
                if alpha[j] + end[j] > m:
                    m = alpha[j] + end[j]
            s = 0.0
            for j in range(Tn):
                s += np.exp(alpha[j] + end[j] - m)
            denom[b] = m + np.log(s)
        return denom

    _HAVE_NUMBA = True
except Exception:
    _HAVE_NUMBA = False


def _logsumexp(a, axis):
    m = np.max(a, axis=axis, keepdims=True)
    return (m + np.log(np.sum(np.exp(a - m), axis=axis, keepdims=True))).squeeze(axis)


def _host_crf(feats, tags, mask, b_out, start_trans, end_trans, transitions):
    feats = feats + np.asarray(b_out, np.float32)[None, None, :]
    trans = np.asarray(transitions, np.float32)
    start = np.asarray(start_trans, np.float32)
    end = np.asarray(end_trans, np.float32)
    maskT = mask.T.astype(np.float32)       # [L, B]
    tagsT = tags.T                          # [L, B]
    em = np.take_along_axis(feats, tagsT[:, :, None], axis=2)[..., 0]  # [L, B]
    score = start[tagsT[0]] + em[0]
    tr = trans[tagsT[:-1], tagsT[1:]]
    score = score + ((tr + em[1:]) * maskT[1:]).sum(axis=0)
    last = mask.sum(axis=1).astype(np.int64) - 1
    last_tags = np.take_along_axis(tags, last[:, None], axis=1)[:, 0]
    score = score + end[last_tags]

    if _HAVE_NUMBA:
        denom = _crf_denom_nb(
            np.ascontiguousarray(feats, np.float32),
            np.exp(np.asarray(transitions, np.float64)),
            start.astype(np.float64), end.astype(np.float64),
            np.ascontiguousarray(maskT, np.float64))
    else:
        alpha = start[None, :] + feats[0]
        trb = trans[None, :, :]
        for t in range(1, L):
            tmp = alpha[:, :, None] + trb
            tmp += feats[t][:, None, :]
            m = tmp.max(axis=1)
            np.exp(tmp - m[:, None, :], out=tmp)
            nxt = m + np.log(tmp.sum(axis=1))
            alpha = np.where(maskT[t][:, None] > 0, nxt, alpha)
        denom = _logsumexp(alpha + end[None, :], axis=1)
    llh = score - denom
    return -(llh.sum() / maskT.sum())


_DEBUG_T = bool(os.environ.get("BASSK_TIME"))


def _tick(label, t0):
    if _DEBUG_T:
        t1 = time.perf_counter()
        print(f"[k] {label}: {(t1 - t0) * 1e3:.1f}ms", flush=True)
    return time.perf_counter()


def kernel(sentence, tags, mask, emb, w_ih_f, w_hh_f, b_f,
           w_ih_b, w_hh_b, b_b, w_out, b_out,
           start_trans, end_trans, transitions):
    t0 = time.perf_counter()
    sentence = np.asarray(sentence)
    tags = np.asarray(tags)
    mask = np.asarray(mask)

    if "nc" not in _state:
        _state["nc"] = _build_program()
        _state["runner"] = _make_runner(_state["nc"])
        _state["needs_warmup"] = True
    runner = _state["runner"]

    t0 = _tick("setup", t0)
    dev = _ensure_weights(runner, {
        "emb": np.asarray(emb), "w_ih_f": np.asarray(w_ih_f),
        "w_hh_f": np.asarray(w_hh_f), "b_f": np.asarray(b_f),
        "w_ih_b": np.asarray(w_ih_b), "w_hh_b": np.asarray(w_hh_b),
        "b_b": np.asarray(b_b), "w_out": np.asarray(w_out),
    })

    t0 = _tick("weights", t0)
    # ---- per-call: token indices, t-major, reversed for backward cores ----
    # idx[p, j] = token id at t-major position j*128 + p
    idx_all = np.empty((8, 128, JB), np.int32)
    for c in range(8):
        sl = sentence[(c % 4) * BL:(c % 4) * BL + BL]   # [BL, L]
        tm = sl.T if c < 4 else sl.T[::-1]              # [L, BL] t-major
        flat = np.ascontiguousarray(tm).reshape(-1)
        idx_all[c] = flat.reshape(JB, 128).T
    idx_concat = idx_all.reshape(8 * 128, JB)

    t0 = _tick("idxprep", t0)
    args = {"idxs": idx_concat, **dev}
    ordered = [args[n] for n in runner["in_names"]]

    def _dispatch():
        zeros = [np.zeros((8 * s[0], *s[1:]), d)
                 for s, d in runner["zero_shapes"]]
        outs = runner["fn"](*ordered, *zeros)
        return [np.asarray(o) for o in outs]

    if _state.pop("needs_warmup", False):
        for _ in range(3):
            _dispatch()
    outs = _dispatch()
    feats_cat = outs[runner["out_names"].index("feats")]
    feats_cat = (feats_cat.view(np.uint16).astype(np.uint32) << np.uint32(16)).view(np.float32)
    t0 = _tick("jit+fetch", t0)

    # ---- assemble full feats [L, B, T] ----
    f = feats_cat.reshape(8, T, L, BL).transpose(0, 2, 3, 1)  # [8, L, BL, T]
    feats_full = np.empty((L, B, T), np.float32)
    for s in range(4):
        feats_full[:, s * BL:(s + 1) * BL, :] = f[s] + f[4 + s, ::-1]

    t0 = _tick("assemble", t0)
    loss = _host_crf(feats_full, tags, mask, b_out, start_trans, end_trans,
                     transitions)
    t0 = _tick("crf", t0)
    return np.float32(loss)
